# revision 1
# baseline (speedup 1.0000x reference)
"""Trainium2 Bass kernel for nn_FeatureEmbedding (4-layer 3x3 conv CNN
with LeakyReLU + sinusoidal positional-encoding add).

Strategy
--------
Data-parallel over the batch dim: 32 batches x 12 frames = 384 images;
each of the 8 NeuronCores processes 48 images (4 batches).

Per image, the whole layer chain runs out of SBUF using an UNPADDED
pitch-64 layout (activations stored [C, 64*64] contiguous). This keeps
every DMA contiguous (a handful of large descriptors instead of
hundreds of 128-byte ones — measured ~9 ms/call faster than the padded
variant) and every ScalarE drain contiguous:

  - Input: 9 flat-shifted copies of the image are DMAed into a
    [45, 4096] bf16 buffer (shift s=(kh-1)*64+(kw-1) in flat pixel
    space, clipped), folding layer 1's (kh, kw, cin) taps into the
    contraction dim -> one K=45 matmul per 512-pixel tile. Column-wrap
    garbage (1 column for each kw!=1 copy) is zeroed by tiny VectorE
    memsets; row spill-over stays zero from a one-time memset.
  - Layers 2-4 are shift-GEMM over zero-padded [C, 66*66] activation
    buffers: for each 512-pixel output tile the 9 taps accumulate into
    one PSUM bank via strided (full-window) moving access patterns.
    ScalarE's strided drains into the padded interiors measured free;
    only small strided DMAs were expensive, and there are none left.
  - Matmuls are tap-outer (same stationary weights for 8 consecutive
    matmuls across the 8 PSUM banks) so weight loads amortize; inputs
    are bf16 (host RNE cast), fp32 accumulation in PSUM.
  - ScalarE drains each bank with fused Lrelu(psum + bias) into the
    next layer's buffer; layer 4 goes to an f32 buffer, VectorE adds
    the per-(t, channel) positional-encoding scalar, DMA to DRAM.

Two buffer sets alternate between images so DMA/PE/ACT pipeline across
images. The conv weights are tiny and pre-marshaled on the host into
the [K, M] stationary layouts the PE wants; the PE table (pure
function of shapes) is precomputed on the host and passed in.
"""

import numpy as np

import concourse.bass as bass
import concourse.bacc as bacc
import concourse.mybir as mybir
import concourse.tile as tile

F32 = mybir.dt.float32
BF16 = mybir.dt.bfloat16
AF = mybir.ActivationFunctionType

N_CORES = 8
B, T, CIN, H, W = 32, 12, 5, 64, 64
# layer-1 folded-K layout: kw-major copy blocks so the wrap-scrub memsets
# start at partition 0 (kw=0 block) and partition 32 (kw=2 block) — DVE
# ops must start at a 0/32/64/96 partition. Partitions 30-31 are dummy
# zero rows (weights zeroed there too).
KWBASE = {0: 0, 1: 15, 2: 32}
K1 = 47
CH = [64, 128, 128, 128]
NPIX = H * W           # 4096
PITCH = W + 2          # 66 (padded row pitch for h buffers)
PAD = PITCH * PITCH    # 4356
NTILE = 8              # 512-pixel output tiles per image
RPT = H // NTILE       # 8 rows per tile
TILEPIX = RPT * W      # 512
ALPHA = 0.01           # LeakyReLU negative slope

TAPS = [(kh, kw) for kh in range(3) for kw in range(3)]


def _build(nimg: int):
    """Build the per-core Bass program (SPMD: same program on all cores)."""
    nc = bacc.Bacc("TRN2", target_bir_lowering=False, debug=False)

    # x and weights are pre-cast to bf16 on the host, so all DMAs are
    # plain copies with no cast step.
    xin = nc.dram_tensor("xin", [nimg, CIN, NPIX], BF16, kind="ExternalInput")
    w1d = nc.dram_tensor("w1", [K1, CH[0]], BF16, kind="ExternalInput")
    w2pd = nc.dram_tensor("w2p", [2 * CH[0], 3 * CH[1]], BF16,
                          kind="ExternalInput")
    w2sd = nc.dram_tensor("w2s", [CH[0], 3 * CH[1]], BF16,
                          kind="ExternalInput")
    w3d = nc.dram_tensor("w3", [CH[1], 9 * CH[2]], BF16, kind="ExternalInput")
    w4d = nc.dram_tensor("w4", [CH[2], 9 * CH[3]], BF16, kind="ExternalInput")
    b1d = nc.dram_tensor("b1", [CH[0], 1], F32, kind="ExternalInput")
    b2d = nc.dram_tensor("b2", [CH[1], 1], F32, kind="ExternalInput")
    b3d = nc.dram_tensor("b3", [CH[2], 1], F32, kind="ExternalInput")
    b4d = nc.dram_tensor("b4", [CH[3], 1], F32, kind="ExternalInput")
    ped = nc.dram_tensor("pe", [CH[3], T], F32, kind="ExternalInput")
    outd = nc.dram_tensor("out", [nimg, CH[3], NPIX], F32,
                          kind="ExternalOutput")

    with tile.TileContext(nc) as tc:
        with (
            tc.tile_pool(name="wpool", bufs=1) as wp,
            tc.tile_pool(name="bpool", bufs=1) as bp,
            tc.tile_pool(name="psum", bufs=8, space="PSUM") as pp,
        ):
            # --- constants ---
            w1s = wp.tile([K1, CH[0]], BF16)
            nc.sync.dma_start(out=w1s, in_=w1d[:, :])
            w2ps = wp.tile([2 * CH[0], 3 * CH[1]], BF16)
            nc.sync.dma_start(out=w2ps, in_=w2pd[:, :])
            w2ss = wp.tile([CH[0], 3 * CH[1]], BF16)
            nc.sync.dma_start(out=w2ss, in_=w2sd[:, :])
            w3s = wp.tile([CH[1], 9 * CH[2]], BF16)
            nc.sync.dma_start(out=w3s, in_=w3d[:, :])
            w4s = wp.tile([CH[2], 9 * CH[3]], BF16)
            nc.sync.dma_start(out=w4s, in_=w4d[:, :])
            b1s = wp.tile([CH[0], 1], F32)
            nc.sync.dma_start(out=b1s, in_=b1d[:, :])
            b2s = wp.tile([CH[1], 1], F32)
            nc.sync.dma_start(out=b2s, in_=b2d[:, :])
            b3s = wp.tile([CH[2], 1], F32)
            nc.sync.dma_start(out=b3s, in_=b3d[:, :])
            b4s = wp.tile([CH[3], 1], F32)
            nc.sync.dma_start(out=b4s, in_=b4d[:, :])
            pes = wp.tile([CH[3], T], F32)
            nc.sync.dma_start(out=pes, in_=ped[:, :])

            # --- persistent activation buffers, double-buffered ---
            sets = []
            for s in range(2):
                x9 = bp.tile([K1, NPIX], BF16, name=f"x9_{s}")
                # h1 holds copy A (parts 0-63) and copy B (parts 64-127,
                # shifted one element left in flat padded space; the wrap
                # lands only on halo zeros, so one contiguous SBUF->SBUF
                # DMA produces an exact shifted-padded copy for pairing
                # layer 2's kw in {0,1} taps into K=128 matmuls)
                h1 = bp.tile([2 * CH[0], PAD], BF16, name=f"h1_{s}")
                h2 = bp.tile([CH[1], PAD], BF16, name=f"h2_{s}")
                h3 = bp.tile([CH[2], PAD], BF16, name=f"h3_{s}")
                h4 = bp.tile([CH[3], NPIX], F32, name=f"h4_{s}")
                # one-time zero: x9's clipped-shift spill-over and the
                # h buffers' halos are never rewritten, so they must
                # start (and stay) 0
                for buf in (x9, h1, h2, h3):
                    nc.vector.memset(buf.bitcast(mybir.dt.uint16), 0.0)
                sets.append((x9, h1, h2, h3, h4))

            for img in range(nimg):
                x9, h1, h2, h3, h4 = sets[img % 2]
                t = img % T
                x9v = x9.rearrange("p (r c) -> p r c", c=W)
                h1v = h1.rearrange("p (r c) -> p r c", c=PITCH)
                h2v = h2.rearrange("p (r c) -> p r c", c=PITCH)
                h3v = h3.rearrange("p (r c) -> p r c", c=PITCH)

                # 1) input: 9 flat-shifted contiguous copies + wrap scrub
                for kh in range(3):
                    for kw in range(3):
                        base = KWBASE[kw] + kh * CIN
                        s = (kh - 1) * W + (kw - 1)
                        d0 = max(0, -s)
                        d1 = NPIX - max(0, s)
                        nc.sync.dma_start(
                            out=x9[base:base + CIN, d0:d1],
                            in_=xin[img, :, d0 + s:d1 + s])
                # wrap-around garbage: col 0 of the kw=0 block, col 63 of
                # the kw=2 block
                nc.vector.memset(
                    x9v[0:15, :, 0:1].bitcast(mybir.dt.uint16), 0.0)
                nc.vector.memset(
                    x9v[32:47, :, W - 1:W].bitcast(mybir.dt.uint16), 0.0)

                # 2) layer 1: one K=45 matmul per tile, contiguous rhs
                ps1 = [pp.tile([CH[0], TILEPIX], F32, name=f"ps1_{img}_{j}",
                               tag="ps") for j in range(NTILE)]
                for j in range(NTILE):
                    nc.tensor.matmul(
                        ps1[j], w1s, x9[:, j * TILEPIX:(j + 1) * TILEPIX],
                        start=True, stop=True)
                for j in range(NTILE):
                    r0 = j * RPT
                    nc.scalar.activation(
                        h1v[0:CH[0], 1 + r0:1 + r0 + RPT, 1:1 + W], ps1[j],
                        AF.Lrelu, bias=b1s[:, 0:1], scale=1.0, alpha=ALPHA)

                nc.sync.dma_start(out=h1[CH[0]:2 * CH[0], 0:PAD - 1],
                                  in_=h1[0:CH[0], 1:PAD])

                # 3) layer 2: 3 paired taps (kw 0+1, K=128) + 3 singles
                # (kw=2, K=64), accumulating into one bank per tile
                ps2 = [pp.tile([CH[1], TILEPIX], F32, name=f"ps2_{img}_{j}",
                               tag="ps") for j in range(NTILE)]
                for kh in range(3):
                    for j in range(NTILE):
                        r0 = j * RPT
                        nc.tensor.matmul(
                            ps2[j], w2ss[:, kh * CH[1]:(kh + 1) * CH[1]],
                            h1v[0:CH[0], r0 + kh:r0 + kh + RPT, 2:2 + W],
                            start=(kh == 0), stop=False)
                for kh in range(3):
                    for j in range(NTILE):
                        r0 = j * RPT
                        nc.tensor.matmul(
                            ps2[j], w2ps[:, kh * CH[1]:(kh + 1) * CH[1]],
                            h1v[:, r0 + kh:r0 + kh + RPT, 0:W],
                            start=False, stop=(kh == 2))
                for j in range(NTILE):
                    r0 = j * RPT
                    nc.scalar.activation(
                        h2v[:, 1 + r0:1 + r0 + RPT, 1:1 + W], ps2[j],
                        AF.Lrelu, bias=b2s[:, 0:1], scale=1.0, alpha=ALPHA)

                # 4) layers 3-4: 9-tap shift-GEMM over padded buffers
                for li, (srcv, dstv, wsb, bsb, cout) in enumerate((
                    (h2v, h3v, w3s, b3s, CH[2]),
                    (h3v, None, w4s, b4s, CH[3]),
                )):
                    psl = [pp.tile([cout, TILEPIX], F32,
                                   name=f"ps{li + 2}_{img}_{j}", tag="ps")
                           for j in range(NTILE)]
                    for ti, (kh, kw) in enumerate(TAPS):
                        tap = kh * 3 + kw
                        for j in range(NTILE):
                            r0 = j * RPT
                            nc.tensor.matmul(
                                psl[j],
                                wsb[:, tap * cout:(tap + 1) * cout],
                                srcv[:, r0 + kh:r0 + kh + RPT, kw:kw + W],
                                start=(ti == 0), stop=(ti == 8))
                    for j in range(NTILE):
                        r0 = j * RPT
                        if dstv is not None:
                            nc.scalar.activation(
                                dstv[:, 1 + r0:1 + r0 + RPT, 1:1 + W],
                                psl[j], AF.Lrelu,
                                bias=bsb[:, 0:1], scale=1.0, alpha=ALPHA)
                        else:
                            nc.scalar.activation(
                                h4[:, j * TILEPIX:(j + 1) * TILEPIX],
                                psl[j], AF.Lrelu,
                                bias=bsb[:, 0:1], scale=1.0, alpha=ALPHA)

                # 4) positional encoding add + store
                nc.vector.tensor_scalar_add(h4, h4, pes[:, t:t + 1])
                nc.sync.dma_start(out=outd[img], in_=h4)

    nc.compile()
    return nc


def _pe_table():
    d = np.arange(CH[3])
    d_even = (d // 2) * 2
    tt = np.arange(T, dtype=np.float64)
    arg = tt[:, None] / np.power(10000.0, d_even / CH[3])
    pe = np.where(d % 2 == 0, np.sin(arg), np.cos(arg))  # [T, D]
    return np.ascontiguousarray(pe.T.astype(np.float32))  # [D, T]


def _cast_bf16(a):
    """fp32 -> bf16 (round-to-nearest-even) on the host so on-chip DMAs
    are plain copies."""
    import ml_dtypes
    return np.ascontiguousarray(np.asarray(a, dtype=np.float32)).astype(
        ml_dtypes.bfloat16)


def _w1_layout(w0):
    """[K1, 64] stationary layout: row KWBASE[kw] + kh*5 + cin; dummy
    zero rows at partitions 30-31."""
    out = np.zeros((K1, CH[0]), dtype=np.float32)
    for kw in range(3):
        for kh in range(3):
            out[KWBASE[kw] + kh * CIN:KWBASE[kw] + (kh + 1) * CIN] = \
                w0[:, :, kh, kw].T
    return out


def _w2_pairs(w1):
    """[128, 3*128]: rows 0-63 = tap (kh, 0), rows 64-127 = tap (kh, 1),
    kh-major blocks."""
    a = w1.transpose(1, 2, 3, 0)  # [cin, kh, kw, cout]
    p = np.concatenate([a[:, :, 0, :], a[:, :, 1, :]], axis=0)
    return np.ascontiguousarray(p).reshape(2 * CH[0], 3 * CH[1])


def _prep_consts(w0, b0, w1, b1, w2, b2, w3, b3):
    consts = {
        "w1": _cast_bf16(_w1_layout(np.asarray(w0))),
        "w2p": _cast_bf16(_w2_pairs(np.asarray(w1))),
        "w2s": _cast_bf16(
            np.ascontiguousarray(
                np.asarray(w1).transpose(1, 2, 3, 0)[:, :, 2, :])
            .reshape(CH[0], 3 * CH[1])),
        "w3": _cast_bf16(
            np.asarray(w2).transpose(1, 2, 3, 0).reshape(CH[1], 9 * CH[2])),
        "w4": _cast_bf16(
            np.asarray(w3).transpose(1, 2, 3, 0).reshape(CH[2], 9 * CH[3])),
        "b1": np.ascontiguousarray(np.asarray(b0, dtype=np.float32)
                                   .reshape(CH[0], 1)),
        "b2": np.ascontiguousarray(np.asarray(b1, dtype=np.float32)
                                   .reshape(CH[1], 1)),
        "b3": np.ascontiguousarray(np.asarray(b2, dtype=np.float32)
                                   .reshape(CH[2], 1)),
        "b4": np.ascontiguousarray(np.asarray(b3, dtype=np.float32)
                                   .reshape(CH[3], 1)),
        "pe": _pe_table(),
    }
    return consts


_prog_cache: dict[int, object] = {}


def _get_program(nimg: int):
    if nimg not in _prog_cache:
        _prog_cache[nimg] = _build(nimg)
    return _prog_cache[nimg]


_runner_cache: dict[int, object] = {}


def _get_runner(nimg: int):
    """A reusable jitted SPMD executor for the per-core program (avoids
    re-tracing/re-lowering on every kernel() call)."""
    if nimg in _runner_cache:
        return _runner_cache[nimg]

    import jax
    import jax.numpy as jnp
    from concourse.bass2jax import (
        install_neuronx_cc_hook, partition_id_tensor, _bass_exec_p)
    from jax.sharding import Mesh, PartitionSpec, NamedSharding
    from jax.experimental.shard_map import shard_map

    nc = _get_program(nimg)
    install_neuronx_cc_hook()

    partition_name = (nc.partition_id_tensor.name
                      if nc.partition_id_tensor else None)
    in_names, out_names, out_avals, zero_shapes = [], [], [], []
    for alloc in nc.m.functions[0].allocations:
        if not isinstance(alloc, mybir.MemoryLocationSet):
            continue
        name = alloc.memorylocations[0].name
        if alloc.kind == "ExternalInput":
            if name != partition_name:
                in_names.append(name)
        elif alloc.kind == "ExternalOutput":
            shape = tuple(alloc.tensor_shape)
            dtype = mybir.dt.np(alloc.dtype)
            out_names.append(name)
            out_avals.append(jax.core.ShapedArray(shape, dtype))
            zero_shapes.append((shape, dtype))
    n_params = len(in_names)
    n_outs = len(out_names)
    all_in_names = list(in_names) + list(out_names)
    if partition_name is not None:
        all_in_names.append(partition_name)

    def _body(*args):
        operands = list(args)
        if partition_name is not None:
            operands.append(partition_id_tensor())
        outs = _bass_exec_p.bind(
            *operands,
            out_avals=tuple(out_avals),
            in_names=tuple(all_in_names),
            out_names=tuple(out_names),
            lowering_input_output_aliases=(),
            sim_require_finite=True,
            sim_require_nnan=True,
            nc=nc,
        )
        return tuple(outs)

    devices = jax.devices()[:N_CORES]
    mesh = Mesh(np.asarray(devices), ("core",))
    sh = NamedSharding(mesh, PartitionSpec("core"))
    donate = tuple(range(n_params, n_params + n_outs))
    sharded = jax.jit(
        shard_map(_body, mesh=mesh,
                  in_specs=(PartitionSpec("core"),) * (n_params + n_outs),
                  out_specs=(PartitionSpec("core"),) * n_outs,
                  check_rep=False),
        donate_argnums=donate, keep_unused=True)
    zeros_fn = jax.jit(
        lambda: tuple(
            jnp.zeros((N_CORES * s[0], *s[1:]), d) for s, d in zero_shapes),
        out_shardings=(sh,) * n_outs)

    def run(in_maps):
        concat_in = [
            np.concatenate([np.asarray(in_maps[c][nm])
                            for c in range(N_CORES)], axis=0)
            for nm in in_names
        ]
        dev_in = [jax.device_put(a, sh) for a in concat_in]
        outs = sharded(*dev_in, *zeros_fn())
        oi = out_names.index("out")
        return np.asarray(outs[oi])

    _runner_cache[nimg] = run
    return run


def make_in_maps(x, w0, b0, w1, b1, w2, b2, w3, b3):
    """Shard the full inputs into the 8 per-core input maps."""
    consts = _prep_consts(w0, b0, w1, b1, w2, b2, w3, b3)
    bpc = B // N_CORES  # batches per core
    in_maps = []
    for c in range(N_CORES):
        xs = _cast_bf16(
            np.asarray(x)[c * bpc:(c + 1) * bpc].reshape(
                bpc * T, CIN, NPIX))
        in_maps.append({"xin": xs, **consts})
    return in_maps


def kernel(x, w0, b0, w1, b1, w2, b2, w3, b3):
    nimg = (B // N_CORES) * T
    run = _get_runner(nimg)
    in_maps = make_in_maps(x, w0, b0, w1, b1, w2, b2, w3, b3)
    glob = run(in_maps)  # [8*nimg, 128, 4096]
    bpc = B // N_CORES
    out = glob.reshape(N_CORES * bpc, T, CH[3], H, W).reshape(
        B, T, CH[3], H, W)
    return np.ascontiguousarray(out.astype(np.float32))

